# revision 5
# baseline (speedup 1.0000x reference)
"""Multi-head attention (qk-norm variant) on 8 TRN2 NeuronCores.

Sharding (Megatron-style, per spec hint): core c handles batch b=c//4 and
head-group hg=c%4 (4 of 16 heads). QKV is column-parallel (each core owns its
heads' rows of w_qkv), attention is fully local per (b, head), and the output
projection is row-parallel: each core produces a partial [N, DIM] output which
the host sums per batch (the "unshard" step) and adds b_proj.

Per-core kernel (bf16 compute, fp32 PSUM accumulation). v2 layout notes:
  - xT arrives pre-transposed so the feature dim lies on SBUF partitions.
  - q,k are head-major ([d, tok], 2 heads per 128 partitions). The qkv weights
    are pre-centered on the host (LN mean-subtraction is linear), so the
    projection emits centered values directly.
  - qk-norm rstd is computed WITHOUT the scalar engine: per-token sum-of-
    squares via a ones-block matmul (O2), bounced through DRAM into a
    [128, 2, 16] token-on-partitions layout, then 1/sqrt via a cubic seed +
    3 Newton steps on DVE.  The ACT engine therefore runs Exp only — one
    table load, no exp<->sqrt table thrashing (was ~21us dead ACT time).
  - rstd returns through DRAM with a partition-replicating AP and is applied
    to q,k with one DVE multiply per half-chunk.
  - softmax needs no max-subtraction: after qk-norm scores*scale are in
    [-8, 8], so exp() is safe.
  - v is token-major with a fused ones-column: the PV matmul (M=65) yields
    the softmax denominator as psum row 64 for free.
  - emission order: k01 -> q01 -> first attention unit, so the ~147us exp
    stream (the ACT roofline of this kernel) starts as soon as the input DMA
    lands; v projections and the qkv/LN chains for heads 2,3 fill the PE
    while early units stream exp; PV of unit u-1 is interleaved per key-tile
    into unit u's scores so the last-unit tail is short; the output
    projection shares the "score" PSUM tag (its two 512-wide halves exactly
    fill one [128,2,512] tile).
"""
import numpy as np
import ml_dtypes

import concourse.bass as bass
import concourse.bacc as bacc
import concourse.tile as tile
from concourse import mybir
from concourse.bass_utils import run_bass_kernel_spmd

F32 = mybir.dt.float32
BF16 = mybir.dt.bfloat16
AF = mybir.ActivationFunctionType
ALU = mybir.AluOpType

B, N, DIM = 2, 2048, 1024
H, D = 16, 64
EPS = 1e-5
N_CORES = 8
HPC = 4              # heads per core
HF = HPC * D         # 256 local head features
KT = DIM // 128      # 8 contraction tiles
NT = N // 128        # 16 token tiles
NCH = N // 512       # 4 token chunks
SCALE = D ** -0.5

# rsqrt seed polynomial (relative-error weighted cubic fit on [0.2, 3.5];
# with 3 Newton steps max rel err < 5e-5 on the realistic ms range)
_tt = np.linspace(0.2, 3.5, 4000)
_RC = np.polyfit(_tt, _tt ** -0.5, 3, w=_tt ** 0.5)

# set by test harness to request NTFF profiling
TRACE = False
LAST_EXEC_NS = None
LAST_RESULTS = None

_BUILD_CACHE = {}


def _build(has_qkbias, has_qgamma, has_kgamma, has_qbeta, has_kbeta,
           has_vbias):
    key = (has_qkbias, has_qgamma, has_kgamma, has_qbeta, has_kbeta,
           has_vbias)
    if key in _BUILD_CACHE:
        return _BUILD_CACHE[key]

    nc = bacc.Bacc("TRN2", target_bir_lowering=False, debug=False,
                   num_devices=N_CORES)

    xT_d = nc.dram_tensor("xT", [DIM, N], BF16, kind="ExternalInput")
    # columns ordered [q01 | k01 | q23 | k23] so group g = cols g*128..
    wqkT_d = nc.dram_tensor("wqkT", [DIM, 2 * HF], BF16, kind="ExternalInput")
    wvT_d = nc.dram_tensor("wvT", [DIM, HF], BF16, kind="ExternalInput")
    wpT_d = nc.dram_tensor("wpT", [HF, DIM], BF16, kind="ExternalInput")
    O2_d = nc.dram_tensor("O2", [128, 2], BF16, kind="ExternalInput")
    bqk_d = ones_d = bvT_d = gamma_d = beta_d = None
    if has_qkbias:
        bqk_d = nc.dram_tensor("bqk_cols", [128, 4], F32, kind="ExternalInput")
    if has_vbias:
        bvT_d = nc.dram_tensor("bvT", [1, HF], BF16, kind="ExternalInput")
        ones_d = nc.dram_tensor("ones512", [1, 512], BF16, kind="ExternalInput")
    if has_qgamma or has_kgamma:
        gamma_d = nc.dram_tensor("gamma_cols", [128, 2], F32, kind="ExternalInput")
    if has_qbeta or has_kbeta:
        beta_d = nc.dram_tensor("beta_cols", [128, 2], F32, kind="ExternalInput")
    out_d = nc.dram_tensor("out_partial", [N, DIM], BF16, kind="ExternalOutput")

    with tile.TileContext(nc) as tc:
        with (
            tc.tile_pool(name="persist", bufs=1) as pp,
            tc.tile_pool(name="work", bufs=2) as wp,
            tc.tile_pool(name="psum", bufs=1, space="PSUM") as psp,
            tc.tile_pool(name="dram", bufs=1, space="DRAM") as dp,
        ):
            # ---- persistent SBUF tensors ----
            xT = [pp.tile([128, N], BF16, name=f"xT{i}") for i in range(KT)]
            wqk = [pp.tile([128, 2 * HF], BF16, name=f"wqk{i}") for i in range(KT)]
            wv = [pp.tile([128, HF], BF16, name=f"wv{i}") for i in range(KT)]
            wpj = [pp.tile([128, DIM], BF16, name=f"wpj{i}") for i in range(2)]
            O2 = pp.tile([128, 2], BF16)
            bqk = pp.tile([128, 4], F32) if bqk_d is not None else None
            bvT = pp.tile([1, HF], BF16) if bvT_d is not None else None
            ones512 = pp.tile([1, 512], BF16) if ones_d is not None else None
            gamma_c = pp.tile([128, 2], F32) if gamma_d is not None else None
            beta_c = pp.tile([128, 2], F32) if beta_d is not None else None

            # v token-major with a ones column at index 64 (width 66 keeps the
            # innermost dim even for DVE perf modes)
            v_sb = pp.tile([128, NT, HPC, 66], BF16)
            # q/k head-major, groups g: 0=q01 1=k01 2=q23 3=k23
            qkt = pp.tile([128, 4, N], BF16)
            outT_n = pp.tile([128, 2, N], BF16)   # attn out, head-major

            # DRAM bounce buffers for the rstd chain, [j, c, p] per group
            rs_g = [dp.tile([2, 16, 128], F32, name=f"rs{g}") for g in range(4)]
            rstd_g = [dp.tile([2, 16, 128], BF16, name=f"rstd{g}")
                      for g in range(4)]

            # ---- input DMA, critical tiles first ----
            for i in range(KT):
                nc.sync.dma_start(out=xT[i], in_=xT_d.ap()[i * 128:(i + 1) * 128, :])
                nc.sync.dma_start(out=wqk[i], in_=wqkT_d.ap()[i * 128:(i + 1) * 128, :])
            nc.sync.dma_start(out=O2, in_=O2_d.ap())
            for i in range(KT):
                nc.sync.dma_start(out=wv[i], in_=wvT_d.ap()[i * 128:(i + 1) * 128, :])
            for i in range(2):
                nc.sync.dma_start(out=wpj[i], in_=wpT_d.ap()[i * 128:(i + 1) * 128, :])
            for t, d in [(bqk, bqk_d), (bvT, bvT_d), (ones512, ones_d),
                         (gamma_c, gamma_d), (beta_c, beta_d)]:
                if t is not None:
                    nc.sync.dma_start(out=t, in_=d.ap())

            nc.vector.memset(v_sb[:, :, :, 64:66], 0.0)
            nc.vector.memset(v_sb[:, :, :, 64:65], 1.0)

            def qk_chunk(g, ch):
                """q/k head-major projection + sum-of-squares for group g,
                chunk ch. ms goes straight to the DRAM bounce buffer."""
                csl = slice(ch * 512, (ch + 1) * 512)
                ps_qk = psp.tile([128, 512], F32, tag="misc", bufs=2,
                                 name="ps_qk")
                for kt in range(KT):
                    nc.tensor.matmul(
                        ps_qk,
                        wqk[kt][:, g * 128:(g + 1) * 128],
                        xT[kt][:, csl],
                        start=(kt == 0), stop=(kt == KT - 1))
                if has_qkbias:
                    nc.vector.tensor_scalar_add(
                        qkt[:, g, csl], ps_qk, bqk[:, g:g + 1])
                else:
                    nc.vector.tensor_copy(qkt[:, g, csl], ps_qk)
                sq = wp.tile([128, 512], BF16, tag="sq", bufs=3)
                nc.vector.tensor_mul(sq, qkt[:, g, csl], qkt[:, g, csl])
                ps_ms = psp.tile([128, 512], F32, tag="misc", bufs=2,
                                 name="ps_ms")
                nc.tensor.matmul(ps_ms[0:2, :], O2, sq, start=True, stop=True)
                ms_sb = wp.tile([2, 512], F32, tag="ms", bufs=3)
                nc.vector.tensor_copy(ms_sb, ps_ms[0:2, :])
                # ms[j, tok] -> rs_g[j, c, p] with tok = c*128 + p
                nc.sync.dma_start(
                    out=rs_g[g][:, ch * 4:(ch + 1) * 4, :],
                    in_=ms_sb.rearrange("p (c q) -> p c q", c=4))

            def ln_group(g):
                """rstd = 1/sqrt(ms/64 + eps) for all 2048 tokens x 2 heads of
                group g, computed on DVE in a token-on-partitions layout."""
                nt_in = wp.tile([128, 2, 16], F32, tag="nt_in", bufs=2)
                nc.sync.dma_start(out=nt_in,
                                  in_=rs_g[g].rearrange("j c p -> p j c"))
                x = nt_in.rearrange("p a b -> p (a b)")
                t = wp.tile([128, 32], F32, tag="nt_t", bufs=2)
                nc.vector.tensor_scalar(t, x, 1.0 / D, EPS, ALU.mult, ALU.add)
                y = wp.tile([128, 32], F32, tag="nt_y", bufs=2)
                a = wp.tile([128, 32], F32, tag="nt_a", bufs=2)
                # cubic seed (Horner), clamped
                nc.vector.tensor_scalar(y, t, float(_RC[0]), float(_RC[1]),
                                        ALU.mult, ALU.add)
                nc.vector.tensor_mul(y, y, t)
                nc.vector.tensor_scalar_add(y, y, float(_RC[2]))
                nc.vector.tensor_mul(y, y, t)
                nc.vector.tensor_scalar_add(y, y, float(_RC[3]))
                nc.vector.tensor_scalar(y, y, 0.1, 2.4, ALU.max, ALU.min)
                nt_out = wp.tile([128, 2, 16], BF16, tag="nt_out", bufs=2)
                for it in range(3):
                    nc.vector.tensor_mul(a, y, y)
                    nc.vector.tensor_mul(a, a, t)
                    nc.vector.tensor_scalar(a, a, -0.5, 1.5, ALU.mult, ALU.add)
                    if it < 2:
                        nc.vector.tensor_mul(y, y, a)
                    else:
                        nc.vector.tensor_mul(
                            nt_out.rearrange("p a b -> p (a b)"), y, a)
                nc.sync.dma_start(
                    out=rstd_g[g].rearrange("j c p -> p j c"), in_=nt_out)

            def apply_chunk(g, ch):
                """multiply qkt chunk by its per-token rstd (DRAM-bounced
                broadcast across partitions; DRAM sources may repeat
                partitions)."""
                csl = slice(ch * 512, (ch + 1) * 512)
                rb = wp.tile([128, 512], BF16, tag="rb", bufs=3)
                for j in range(2):
                    row = rstd_g[g][j:j + 1, ch * 4:(ch + 1) * 4, :]
                    row = row.rearrange("j c p -> j (c p)")
                    bc = bass.AP(tensor=row.tensor, offset=row.offset,
                                 ap=[[0, 64]] + list(row.ap[1:]))
                    nc.sync.dma_start(out=rb[64 * j:64 * (j + 1), :], in_=bc)
                nc.vector.tensor_mul(qkt[0:64, g, csl], qkt[0:64, g, csl],
                                     rb[0:64, :])
                nc.vector.tensor_mul(qkt[64:128, g, csl], qkt[64:128, g, csl],
                                     rb[64:128, :])
                is_q = (g % 2 == 0)
                gcol = None
                if is_q and has_qgamma:
                    gcol = gamma_c[:, 0:1]
                elif not is_q and has_kgamma:
                    gcol = gamma_c[:, 1:2]
                bcol = None
                if is_q and has_qbeta:
                    bcol = beta_c[:, 0:1]
                elif not is_q and has_kbeta:
                    bcol = beta_c[:, 1:2]
                if gcol is not None:
                    nc.vector.tensor_scalar_mul(qkt[:, g, csl],
                                                qkt[:, g, csl], gcol)
                if bcol is not None:
                    nc.vector.tensor_scalar_add(qkt[:, g, csl],
                                                qkt[:, g, csl], bcol)

            def qk_group(g):
                for ch in range(NCH):
                    qk_chunk(g, ch)
                ln_group(g)
                for ch in range(NCH):
                    apply_chunk(g, ch)

            def v_feats(tt):
                """v token-major projection for token tile tt."""
                tsl = slice(tt * 128, (tt + 1) * 128)
                ps_v = psp.tile([128, 512], F32, tag="misc", bufs=2,
                                name="ps_v")
                for kt in range(KT):
                    nc.tensor.matmul(
                        ps_v[:, 0:HF], xT[kt][:, tsl], wv[kt],
                        start=(kt == 0),
                        stop=(not has_vbias and kt == KT - 1))
                if has_vbias:
                    nc.tensor.matmul(ps_v[:, 0:HF], ones512[:, 0:128],
                                     bvT, start=False, stop=True)
                nc.vector.tensor_copy(
                    v_sb[:, tt, :, 0:64],
                    ps_v[:, 0:HF].rearrange("p (h d) -> p h d", h=HPC))

            def pv_block(pgq, pqc, pexp):
                """PV matmuls for a whole previous unit (non-interleaved
                fallback used for the trailing unit)."""
                pouts = [psp.tile([65, 512], F32, tag="pvc", bufs=2,
                                  name=f"ps_o{hp}") for hp in range(2)]
                for kt in range(NT):
                    for hp in range(2):
                        h = 2 * pgq + hp
                        nc.tensor.matmul(pouts[hp], v_sb[:, kt, h, 0:65],
                                         pexp[:, kt, hp, :],
                                         start=(kt == 0), stop=(kt == NT - 1))
                return pouts

            def normalize(pgq, pqc, pouts):
                """divide PV psum by the fused denominator row, write outT.
                (reciprocal_approx_fast misreads PSUM sources — stage the
                denominator row through SBUF first)"""
                qsl = slice(pqc * 512, (pqc + 1) * 512)
                for hp in range(2):
                    p0 = hp * 64
                    ps_o = pouts[hp]
                    den = wp.tile([1, 512], F32, tag="den", bufs=3)
                    nc.vector.tensor_copy(den, ps_o[64:65, :])
                    rec = wp.tile([1, 512], F32, tag="rec", bufs=3)
                    nc.vector.reciprocal_approx_fast(rec, den)
                    rb2 = wp.tile([64, 512], F32, tag="rb2", bufs=3)
                    nc.gpsimd.partition_broadcast(rb2, rec)
                    nc.vector.tensor_mul(outT_n[p0:p0 + 64, pgq, qsl],
                                         ps_o[0:64, :], rb2)

            def proj_chunk(qc):
                """output projection; one score-tag psum tile per token tile
                (two 512-wide halves of DIM)."""
                for tt in range(qc * 4, qc * 4 + 4):
                    tsl = slice(tt * 128, (tt + 1) * 128)
                    ps_p = psp.tile([128, 2, 512], F32, tag="score", bufs=2,
                                    name="ps_p")
                    for fn in range(2):
                        fsl = slice(fn * 512, (fn + 1) * 512)
                        for t in range(2):
                            nc.tensor.matmul(ps_p[:, fn, :],
                                             outT_n[:, t, tsl],
                                             wpj[t][:, fsl],
                                             start=(t == 0), stop=(t == 1))
                    ostg = wp.tile([128, DIM], BF16, tag="ostg", bufs=3)
                    nc.vector.tensor_copy(
                        ostg, ps_p.rearrange("p a b -> p (a b)"))
                    nc.sync.dma_start(out=out_d.ap()[tsl, :], in_=ostg)

            def unit(gq, qc, prev):
                """scores+exp for unit (gq, qc); PV of the previous unit is
                interleaved per key-tile so its matmuls ride under this
                unit's exp stream. Returns (gq, qc, exp_pair)."""
                qg, kg = (0, 1) if gq == 0 else (2, 3)
                qsl = slice(qc * 512, (qc + 1) * 512)
                exp_pair = wp.tile([128, NT, 2, 512], BF16, tag="exp",
                                   bufs=2, name="exp_pair")
                pouts = None
                if prev is not None:
                    pgq, pqc, pexp = prev
                    pouts = [psp.tile([65, 512], F32, tag="pvc", bufs=2,
                                      name=f"ps_o{hp}") for hp in range(2)]
                for kt in range(NT):
                    ktsl = slice(kt * 128, (kt + 1) * 128)
                    ps_s = psp.tile([128, 2, 512], F32, tag="score",
                                    bufs=2, name="ps_s")
                    for hp in range(2):
                        p0 = hp * 64
                        nc.tensor.matmul(ps_s[:, hp, :],
                                         qkt[p0:p0 + 64, kg, ktsl],
                                         qkt[p0:p0 + 64, qg, qsl],
                                         start=True, stop=True)
                    nc.scalar.activation(exp_pair[:, kt, :, :], ps_s,
                                         AF.Exp, scale=SCALE)
                    if prev is not None:
                        for hp in range(2):
                            h = 2 * pgq + hp
                            nc.tensor.matmul(
                                pouts[hp], v_sb[:, kt, h, 0:65],
                                pexp[:, kt, hp, :],
                                start=(kt == 0), stop=(kt == NT - 1))
                if prev is not None:
                    normalize(pgq, pqc, pouts)
                return (gq, qc, exp_pair)

            # ---- emission (priority order = emission order) ----
            with nc.named_scope("prologue"):
                qk_group(1)          # k01 first: gates every unit of gq=0
                qk_group(0)          # q01
            with nc.named_scope("attn"):
                u = unit(0, 0, None)
                for tt in range(NT):
                    v_feats(tt)      # PE filler under exp of unit (0,0)
                u = unit(0, 1, u)
                qk_group(2)          # q23, under exp of early units
                u = unit(0, 2, u)
                qk_group(3)          # k23
                u = unit(0, 3, u)
                u = unit(1, 0, u)
                u = unit(1, 1, u)
                proj_chunk(0)
                u = unit(1, 2, u)
                proj_chunk(1)
                u = unit(1, 3, u)
                proj_chunk(2)
                pouts = pv_block(*u)
                normalize(u[0], u[1], pouts)
                proj_chunk(3)

    nc.compile()
    _BUILD_CACHE[key] = nc
    return nc


def _bf16(a):
    return np.ascontiguousarray(a).astype(ml_dtypes.bfloat16)


def kernel(**inputs):
    global LAST_EXEC_NS
    x = np.asarray(inputs["x"], np.float32)
    w_qkv = np.asarray(inputs["w_qkv"], np.float32)
    b_qkv = np.asarray(inputs["b_qkv"], np.float32)
    q_gamma = np.asarray(inputs["q_gamma"], np.float32)
    q_beta = np.asarray(inputs["q_beta"], np.float32)
    k_gamma = np.asarray(inputs["k_gamma"], np.float32)
    k_beta = np.asarray(inputs["k_beta"], np.float32)
    w_proj = np.asarray(inputs["w_proj"], np.float32)
    b_proj = np.asarray(inputs["b_proj"], np.float32)

    has_qkbias = bool(np.any(b_qkv[0:2 * DIM] != 0.0))
    has_qgamma = not bool(np.all(q_gamma == 1.0))
    has_kgamma = not bool(np.all(k_gamma == 1.0))
    has_qbeta = bool(np.any(q_beta != 0.0))
    has_kbeta = bool(np.any(k_beta != 0.0))
    has_vbias = bool(np.any(b_qkv[2 * DIM:3 * DIM] != 0.0))
    nc = _build(has_qkbias, has_qgamma, has_kgamma, has_qbeta, has_kbeta,
                has_vbias)

    # shared constants
    Cd = np.eye(D, dtype=np.float32) - 1.0 / D   # centering matrix (folded
    O2 = np.zeros((128, 2), np.float32)          # into the qkv weights below)
    O2[:D, 0] = 1.0
    O2[D:, 1] = 1.0
    gamma_cols = np.stack([np.tile(q_gamma, 2), np.tile(k_gamma, 2)],
                          axis=1).astype(np.float32)
    ones512 = np.ones((1, 512), np.float32)
    beta_cols = np.stack([np.tile(q_beta, 2), np.tile(k_beta, 2)],
                         axis=1).astype(np.float32)
    # pre-center the q/k projection weights and biases per head:
    # LN(Wx+b) centering is linear, so fold (I - J/64) into W and b
    w_qkv = w_qkv.copy()
    b_qkv = b_qkv.copy()
    for h in range(2 * H):            # 16 q heads then 16 k heads
        rs = slice(h * D, (h + 1) * D)
        w_qkv[rs] = Cd @ w_qkv[rs]
        b_qkv[rs] = Cd @ b_qkv[rs]

    in_maps = []
    for c in range(N_CORES):
        b, hg = divmod(c, 4)
        rows = slice(hg * HF, (hg + 1) * HF)
        q_l = w_qkv[0 * DIM:1 * DIM][rows]           # [256, 1024]
        k_l = w_qkv[1 * DIM:2 * DIM][rows]
        v_l = w_qkv[2 * DIM:3 * DIM][rows]
        bq_l = b_qkv[0 * DIM:1 * DIM][rows]
        bk_l = b_qkv[1 * DIM:2 * DIM][rows]
        bv_l = b_qkv[2 * DIM:3 * DIM][rows]
        # group order [q01 | k01 | q23 | k23]
        wqk_rows = np.concatenate([q_l[:128], k_l[:128],
                                   q_l[128:], k_l[128:]], 0)
        bqk_cols = np.stack([bq_l[:128], bk_l[:128], bq_l[128:], bk_l[128:]],
                            axis=1).astype(np.float32)
        m = {
            "xT": _bf16(x[b].T),                          # [1024, 2048]
            "wqkT": _bf16(wqk_rows.T),                    # [1024, 512]
            "wvT": _bf16(v_l.T),                          # [1024, 256]
            "wpT": _bf16(w_proj[:, rows].T),              # [256, 1024]
            "O2": _bf16(O2),
        }
        if has_qkbias:
            m["bqk_cols"] = bqk_cols
        if has_vbias:
            m["bvT"] = _bf16(bv_l[None, :])
            m["ones512"] = _bf16(ones512)
        if has_qgamma or has_kgamma:
            m["gamma_cols"] = gamma_cols
        if has_qbeta or has_kbeta:
            m["beta_cols"] = beta_cols
        in_maps.append(m)

    res = run_bass_kernel_spmd(nc, in_maps, core_ids=list(range(N_CORES)),
                               trace=TRACE)
    LAST_EXEC_NS = res.exec_time_ns
    globals()["LAST_RESULTS"] = res

    out = np.zeros((B, N, DIM), np.float32)
    for c in range(N_CORES):
        out[c // 4] += np.asarray(res.results[c]["out_partial"], np.float32)
    out += b_proj[None, None, :]
    return out
